# revision 1
# baseline (speedup 1.0000x reference)
"""Cross-attention layer (Q/KV proj + 4-head attention + out-proj + residual + LayerNorm)
as an 8-core SPMD Bass/Tile kernel for Trainium2.

Sharding: data-parallel over (batch b, query-half). Core c handles batch b=c//2,
query rows [(c%2)*1024, (c%2+1)*1024). Each core recomputes K/V for its batch
(duplicated across the 2 cores sharing a batch — cheap) so no collectives are needed.

Per-core pipeline (all shapes per core):
  - PE-transpose q_in [1024,256] and k_in [2048,256] into C-major layout
  - Q^T = Wq^T @ q_in^T   [256,1024]  (f32r matmuls, bf16 output)
  - K^T = Wk^T @ k_in^T   [256,2048]  (f32r, bf16 out)
  - V   = k_in @ Wv       [2048,256]  (f32r, bf16 out, +ones column for row-sums)
  - scores^T = K_h Q_h^T per head in PSUM (bf16, K=64), exp on ScalarE with
    fused *0.125 scale (no max-subtraction: scores ~ N(0,1), fp32-safe)
  - attn = P V via PSUM-accumulated matmuls; softmax denominator comes free from
    the ones column; normalize on VectorE during PSUM eviction
  - attn^T via PE transpose, out-proj (bf16), +residual, LayerNorm epilogue
"""
import sys

sys.path.insert(0, "/opt/trn_rl_repo")

from contextlib import ExitStack

import numpy as np

import concourse.bacc as bacc
import concourse.bass as bass
import concourse.tile as tile
from concourse import mybir
from concourse.bass_utils import run_bass_kernel_spmd
from concourse.masks import make_identity

P = 128
C = 256          # embed dim
H = 4            # heads
D = 64           # head dim
NQ = 1024        # query rows per core
NK = 2048        # kv rows per core
NQT = NQ // P    # 8 query tiles
NKT = NK // P    # 16 kv tiles
NQC = 4          # attention query chunks
QC = NQ // NQC   # 256 q rows per chunk
SCALE = D ** -0.5
EPS = 1e-5

F32 = mybir.dt.float32
F32R = mybir.dt.float32r
BF16 = mybir.dt.bfloat16
EXP = mybir.ActivationFunctionType.Exp
SQRT = mybir.ActivationFunctionType.Sqrt
ADD = mybir.AluOpType.add
SUB = mybir.AluOpType.subtract
MULT = mybir.AluOpType.mult

N_CORES = 8


def _bcast(src_1d: bass.AP, nparts: int = P) -> bass.AP:
    """Broadcast a 1-D DRAM AP across `nparts` partitions (stride-0 partition dim)."""
    return bass.AP(
        tensor=src_1d.tensor,
        offset=src_1d.offset,
        ap=[[0, nparts]] + [list(d) for d in src_1d.ap],
    )


def build_program():
    nc = bacc.Bacc(
        "TRN2",
        target_bir_lowering=False,
        debug=False,
        enable_asserts=True,
        num_devices=N_CORES,
    )

    q_d = nc.dram_tensor("q", [NQ, C], F32, kind="ExternalInput").ap()
    kin_d = nc.dram_tensor("kin", [NK, C], F32, kind="ExternalInput").ap()
    wq_d = nc.dram_tensor("Wq", [C, C], F32, kind="ExternalInput").ap()
    bq_d = nc.dram_tensor("bq", [C], F32, kind="ExternalInput").ap()
    wkv_d = nc.dram_tensor("Wkv", [C, 2 * C], F32, kind="ExternalInput").ap()
    bkv_d = nc.dram_tensor("bkv", [2 * C], F32, kind="ExternalInput").ap()
    wo_d = nc.dram_tensor("Wo", [C, C], F32, kind="ExternalInput").ap()
    bo_d = nc.dram_tensor("bo", [C], F32, kind="ExternalInput").ap()
    gamma_d = nc.dram_tensor("gamma", [C], F32, kind="ExternalInput").ap()
    beta_d = nc.dram_tensor("beta", [C], F32, kind="ExternalInput").ap()
    y_d = nc.dram_tensor("y", [NQ, C], F32, kind="ExternalOutput").ap()

    with tile.TileContext(nc) as tc:
        with ExitStack() as ctx:
            _body(ctx, tc, q_d, kin_d, wq_d, bq_d, wkv_d, bkv_d, wo_d, bo_d,
                  gamma_d, beta_d, y_d)

    nc.compile()
    return nc


def _body(ctx, tc, q_d, kin_d, wq_d, bq_d, wkv_d, bkv_d, wo_d, bo_d,
          gamma_d, beta_d, y_d):
    nc = tc.nc

    # ---- pools -------------------------------------------------------------
    persist = ctx.enter_context(tc.tile_pool(name="persist", bufs=1))
    ldpool = ctx.enter_context(tc.tile_pool(name="ld", bufs=3))
    ptpool = ctx.enter_context(tc.tile_pool(name="pt", bufs=3))
    small = ctx.enter_context(tc.tile_pool(name="small", bufs=4))
    ypool = ctx.enter_context(tc.tile_pool(name="yout", bufs=3))
    spsum = ctx.enter_context(tc.tile_pool(name="spsum", bufs=2, space="PSUM"))
    wpsum = ctx.enter_context(tc.tile_pool(name="wpsum", bufs=3, space="PSUM"))
    apsum = ctx.enter_context(tc.tile_pool(name="apsum", bufs=1, space="PSUM"))

    # ---- constants / weights ----------------------------------------------
    ident_f = persist.tile([P, P], F32, tag="identf")
    make_identity(nc, ident_f)

    wq_f = persist.tile([P, 2, C], F32, tag="wqf")
    nc.sync.dma_start(out=wq_f, in_=wq_d.rearrange("(j p) m -> p j m", p=P))
    wq_sb = persist.tile([P, 2, C], F32R, tag="wq")
    nc.vector.tensor_copy(out=wq_sb, in_=wq_f)
    wkv_f = persist.tile([P, 2, 2 * C], F32, tag="wkvf")
    nc.sync.dma_start(out=wkv_f, in_=wkv_d.rearrange("(j p) m -> p j m", p=P))
    wkv_sb = persist.tile([P, 2, 2 * C], F32R, tag="wkv")
    nc.vector.tensor_copy(out=wkv_sb, in_=wkv_f)
    wo_f = persist.tile([P, 2, C], F32, tag="wof")
    nc.sync.dma_start(out=wo_f, in_=wo_d.rearrange("(j p) m -> p j m", p=P))
    wo_sb = persist.tile([P, 2, C], BF16, tag="wo")
    nc.vector.tensor_copy(out=wo_sb, in_=wo_f)

    bq_sb = persist.tile([P, 2], F32, tag="bq")
    nc.sync.dma_start(out=bq_sb, in_=bq_d.rearrange("(j p) -> p j", p=P))
    bk_sb = persist.tile([P, 2], F32, tag="bk")
    nc.sync.dma_start(out=bk_sb, in_=bkv_d[0:C].rearrange("(j p) -> p j", p=P))
    bv_bc = persist.tile([P, C], F32, tag="bv")
    nc.sync.dma_start(out=bv_bc, in_=_bcast(bkv_d[C:2 * C]))
    bo_bc = persist.tile([P, C], F32, tag="bo")
    nc.sync.dma_start(out=bo_bc, in_=_bcast(bo_d))
    gamma_bc = persist.tile([P, C], F32, tag="gamma")
    nc.sync.dma_start(out=gamma_bc, in_=_bcast(gamma_d))
    beta_bc = persist.tile([P, C], F32, tag="beta")
    nc.sync.dma_start(out=beta_bc, in_=_bcast(beta_d))

    # ---- persistent activations -------------------------------------------
    qnat = persist.tile([P, NQT, C], F32, tag="qnat")
    qinT = persist.tile([P, 2, NQ], F32R, tag="qinT")
    kinT = persist.tile([P, 2, NK], F32R, tag="kinT")
    QT = persist.tile([P, 2, NQ], BF16, tag="QT")
    KT = persist.tile([P, 2, NK], BF16, tag="KT")
    Vb = persist.tile([P, NKT, H, D + 1], BF16, tag="Vb")
    ATTN = persist.tile([P, NQT, C], F32, tag="ATTN")
    attnT = persist.tile([P, 2, NQ], BF16, tag="attnT")
    tres = persist.tile([P, NQT, C], F32, tag="tres")
    mvall = persist.tile([P, NQT, 2], F32, tag="mvall")

    # ones column of V' (softmax denominator trick)
    nc.vector.memset(Vb[:, :, :, D:D + 1], 1.0)

    # ---- load q, transpose, Q projection ----------------------------------
    for qt in range(NQT):
        qeng = nc.sync if qt % 2 == 0 else nc.gpsimd
        qeng.dma_start(out=qnat[:, qt, :], in_=q_d[qt * P:(qt + 1) * P, :])
        tp = wpsum.tile([P, C], F32, tag="work")
        for j in range(2):
            nc.tensor.transpose(tp[:, j * P:(j + 1) * P],
                                qnat[:, qt, j * P:(j + 1) * P], ident_f)
        nc.vector.tensor_copy(
            out=qinT[:, :, qt * P:(qt + 1) * P],
            in_=tp.rearrange("p (j q) -> p j q", j=2))

    for j2 in range(2):            # output d-chunk
        for qh in range(2):        # 512-wide q chunk
            ps = wpsum.tile([P, 512], F32, tag="work")
            for jc in range(2):    # contraction c-chunk
                nc.tensor.matmul(
                    ps,
                    wq_sb[:, jc, j2 * P:(j2 + 1) * P],
                    qinT[:, jc, qh * 512:(qh + 1) * 512],
                    start=(jc == 0), stop=(jc == 1))
            nc.vector.tensor_scalar(
                out=QT[:, j2, qh * 512:(qh + 1) * 512], in0=ps,
                scalar1=bq_sb[:, j2:j2 + 1], scalar2=None, op0=ADD)

    # ---- load k, transpose, K/V projections -------------------------------
    # Interleave k transposes with K-projection per 512-wide chunk so the first
    # QK^T tiles become runnable long before the whole k phase finishes.
    for kh in range(4):
        for kt in range(kh * 4, (kh + 1) * 4):
            klt = ldpool.tile([P, C], F32, tag="kld")
            nc.gpsimd.dma_start(out=klt, in_=kin_d[kt * P:(kt + 1) * P, :])
            tp = wpsum.tile([P, C], F32, tag="work")
            for j in range(2):
                nc.tensor.transpose(tp[:, j * P:(j + 1) * P],
                                    klt[:, j * P:(j + 1) * P], ident_f)
            nc.vector.tensor_copy(
                out=kinT[:, :, kt * P:(kt + 1) * P],
                in_=tp.rearrange("p (j q) -> p j q", j=2))
        for j2 in range(2):
            ps = wpsum.tile([P, 512], F32, tag="work")
            for jc in range(2):
                nc.tensor.matmul(
                    ps,
                    wkv_sb[:, jc, j2 * P:(j2 + 1) * P],
                    kinT[:, jc, kh * 512:(kh + 1) * 512],
                    start=(jc == 0), stop=(jc == 1))
            nc.vector.tensor_scalar(
                out=KT[:, j2, kh * 512:(kh + 1) * 512], in0=ps,
                scalar1=bk_sb[:, j2:j2 + 1], scalar2=None, op0=ADD)

    for kt in range(NKT):
        ps = wpsum.tile([P, C], F32, tag="work")
        for jc in range(2):
            nc.tensor.matmul(
                ps,
                kinT[:, jc, kt * P:(kt + 1) * P],
                wkv_sb[:, jc, C:2 * C],
                start=(jc == 0), stop=(jc == 1))
        nc.vector.tensor_tensor(
            out=Vb[:, kt, :, 0:D],
            in0=ps.rearrange("p (h d) -> p h d", h=H),
            in1=bv_bc.rearrange("p (h d) -> p h d", h=H),
            op=ADD)

    # ---- attention ---------------------------------------------------------
    # Heads are laid out in scores/PT slots in order [0, 2, 1, 3] so that the two
    # matmuls sharing a PSUM bank use the same partition offset: concurrent PE
    # matmuls from different row groups writing one bank are a fatal PSUM
    # collision on TRN2.
    HPERM = [0, 2, 1, 3]
    for qc in range(NQC):
        pt = ptpool.tile([P, NKT, H, QC], BF16, tag="pt")
        for kt in range(NKT):
            s = spsum.tile([P, H, QC], F32, tag="scores")
            for m in range(H):
                h = HPERM[m]
                j2, po = h // 2, (h % 2) * D
                nc.tensor.matmul(
                    s[:, m, :],
                    KT[po:po + D, j2, kt * P:(kt + 1) * P],
                    QT[po:po + D, j2, qc * QC:(qc + 1) * QC],
                    start=True, stop=True)
            nc.scalar.activation(out=pt[:, kt, :, :], in_=s, func=EXP, scale=SCALE)

        for h in range(H):
            for ql in range(2):
                qt = qc * 2 + ql
                av = apsum.tile([P, D + 1], F32, tag="av")
                for kt in range(NKT):
                    nc.tensor.matmul(
                        av,
                        pt[:, kt, HPERM[h], ql * P:(ql + 1) * P],
                        Vb[:, kt, h, :],
                        start=(kt == 0), stop=(kt == NKT - 1))
                rec = small.tile([P, 1], F32, tag="rec")
                nc.vector.reciprocal(rec, av[:, D:D + 1])
                nc.vector.tensor_scalar(
                    out=ATTN[:, qt, h * D:(h + 1) * D], in0=av[:, 0:D],
                    scalar1=rec, scalar2=None, op0=MULT)

        for ql in range(2):
            qt = qc * 2 + ql
            tp = wpsum.tile([P, C], F32, tag="work")
            for j in range(2):
                nc.tensor.transpose(tp[:, j * P:(j + 1) * P],
                                    ATTN[:, qt, j * P:(j + 1) * P], ident_f)
            nc.vector.tensor_copy(
                out=attnT[:, :, qt * P:(qt + 1) * P],
                in_=tp.rearrange("p (j q) -> p j q", j=2))

            yp = wpsum.tile([P, C], F32, tag="work")
            for jc in range(2):
                nc.tensor.matmul(
                    yp,
                    attnT[:, jc, qt * P:(qt + 1) * P],
                    wo_sb[:, jc, :],
                    start=(jc == 0), stop=(jc == 1))
            nc.vector.tensor_tensor(out=tres[:, qt, :], in0=yp,
                                    in1=qnat[:, qt, :], op=ADD)
            nc.vector.tensor_tensor(out=tres[:, qt, :], in0=tres[:, qt, :],
                                    in1=bo_bc, op=ADD)
            bns = small.tile([P, nc.vector.BN_STATS_DIM], F32, tag="bns")
            nc.vector.bn_stats(out=bns, in_=tres[:, qt, :])
            nc.vector.bn_aggr(out=mvall[:, qt, :], in_=bns)

    # ---- LayerNorm epilogue ------------------------------------------------
    eps_t = small.tile([P, 1], F32, tag="eps")
    nc.vector.memset(eps_t, EPS)
    sd = small.tile([P, NQT], F32, tag="sd")
    nc.scalar.activation(out=sd, in_=mvall[:, :, 1], func=SQRT, bias=eps_t)
    rstd = small.tile([P, NQT], F32, tag="rstd")
    nc.vector.reciprocal(rstd, sd)
    for qt in range(NQT):
        yt = ypool.tile([P, C], F32, tag="yt")
        nc.vector.tensor_scalar(
            out=yt, in0=tres[:, qt, :],
            scalar1=mvall[:, qt, 0:1], scalar2=rstd[:, qt:qt + 1],
            op0=SUB, op1=MULT)
        nc.vector.tensor_tensor(out=yt, in0=yt, in1=gamma_bc, op=MULT)
        nc.vector.tensor_tensor(out=yt, in0=yt, in1=beta_bc, op=ADD)
        nc.gpsimd.dma_start(out=y_d[qt * P:(qt + 1) * P, :], in_=yt)


_PROGRAM = None


def _get_program():
    global _PROGRAM
    if _PROGRAM is None:
        _PROGRAM = build_program()
    return _PROGRAM


def kernel(q_in, k_in, Wq, bq, Wkv, bkv, Wo, bo, gamma, beta, _trace=False):
    q_in = np.ascontiguousarray(np.asarray(q_in, np.float32))
    k_in = np.ascontiguousarray(np.asarray(k_in, np.float32))
    weights = {
        "Wq": np.ascontiguousarray(np.asarray(Wq, np.float32)),
        "bq": np.ascontiguousarray(np.asarray(bq, np.float32)),
        "Wkv": np.ascontiguousarray(np.asarray(Wkv, np.float32)),
        "bkv": np.ascontiguousarray(np.asarray(bkv, np.float32)),
        "Wo": np.ascontiguousarray(np.asarray(Wo, np.float32)),
        "bo": np.ascontiguousarray(np.asarray(bo, np.float32)),
        "gamma": np.ascontiguousarray(np.asarray(gamma, np.float32)),
        "beta": np.ascontiguousarray(np.asarray(beta, np.float32)),
    }
    B, NQ_full, _ = q_in.shape

    nc = _get_program()
    in_maps = []
    for c in range(N_CORES):
        b, half = c // 2, c % 2
        in_maps.append({
            "q": np.ascontiguousarray(q_in[b, half * NQ:(half + 1) * NQ, :]),
            "kin": np.ascontiguousarray(k_in[b]),
            **weights,
        })
    res = run_bass_kernel_spmd(nc, in_maps, core_ids=list(range(N_CORES)),
                               trace=_trace)

    out = np.empty((B, NQ_full, C), np.float32)
    for c in range(N_CORES):
        b, half = c // 2, c % 2
        out[b, half * NQ:(half + 1) * NQ, :] = res.results[c]["y"]
    if _trace:
        return out, res
    return out

